# revision 1
# baseline (speedup 1.0000x reference)
"""Multi-head causal attention (B=4, S=2048, D=768, H=4 heads) on 8 TRN2 cores.

Sharding: core c handles batch b = c//2 and head-pair hp = c%2 (heads 2*hp,
2*hp+1).  Each core projects x[b] through its 384-column slice of Wq/Wk/Wv,
runs causal attention for its two heads, and pushes the result through its
384-row slice of Wo.  The host sums the two partial outputs per batch and
adds bo.  This splits every matmul's FLOPs exactly 8 ways with no duplicated
work and needs no device collectives.

Dataflow is kept transposed end-to-end ([feature, seq] layouts) so the kernel
needs zero on-device transposes:
  QT/KT = W^T x^T           [384, S]   (3 chunks of 128 partitions)
  V     = x W               [S, 384]   (16 chunks of 128 partitions, per-head
                                        layout [d0:128 | ones | d128:192] so a
                                        ones column rides along as the softmax
                                        denominator row)
  S^T   = (KT)^T-contract   [k, q]     k on partitions -> softmax sum over k
                                        comes out of the ctx matmul for free
  ctx^T = V^T E             [192+1, q]
  out^T = Wo^T ctx^T        [768, S]
Causal structure: key-tile i (128 rows) x query-tile j (512 cols) blocks with
i > 4j+3 are fully masked and skipped entirely; diagonal blocks get a 0/1
mask multiply after exp.  Scores are O(1) so exp needs no max-subtraction.

Matmul operands are fp16 (PSUM accumulates fp32).
"""

import sys

for _p in ("/opt/trn_rl_repo",):
    if _p not in sys.path:
        sys.path.insert(0, _p)

import numpy as np

S = 2048            # sequence length
D = 768             # model dim
DH = 192            # head dim
DD = 2 * DH         # feature columns per core (2 heads)
P = 128             # partitions
KC = D // P         # 6 contraction chunks over D
MC = DD // P        # 3 chunks over the per-core head dims
QT = 512            # query tile (matmul free dim, one PSUM bank)
NQ = S // QT        # 4 query tiles
NK = S // P         # 16 key tiles
SCALE = 1.0 / float(np.sqrt(DH))

# Per-head slices of the [384 -> 3x128chunk] QT/KT layout, ordered so the two
# K=64 pieces of the two heads land in different PE row groups (base partition
# 0 vs 64) and can overlap in the array.
#   h=0: chunk0 rows 0:128  +  chunk1 rows 0:64
#   h=1: chunk2 rows 0:128  +  chunk1 rows 64:128
HEAD_PIECES = [
    [(0, 0, 128), (1, 0, 64)],
    [(2, 0, 128), (1, 64, 64)],
]

_CACHE = {}


def _build_nc():
    import concourse.bacc as bacc
    import concourse.tile as tile
    from concourse import mybir

    F16 = mybir.dt.float16
    F32 = mybir.dt.float32
    EXP = mybir.ActivationFunctionType.Exp

    nc = bacc.Bacc(None, target_bir_lowering=False)

    xt = nc.dram_tensor("xt", [P, KC, S], F16, kind="ExternalInput")
    wq = nc.dram_tensor("wq", [P, KC, DD], F16, kind="ExternalInput")
    wk = nc.dram_tensor("wk", [P, KC, DD], F16, kind="ExternalInput")
    wv = nc.dram_tensor("wv", [P, KC, DD], F16, kind="ExternalInput")
    wo = nc.dram_tensor("wo", [P, 4, D], F16, kind="ExternalInput")
    bqk = nc.dram_tensor("bqk", [P, 6], F32, kind="ExternalInput")
    bvr = nc.dram_tensor("bvr", [1, DD], F16, kind="ExternalInput")
    msk = nc.dram_tensor("msk", [P, 4, QT], F16, kind="ExternalInput")
    out_t = nc.dram_tensor("out_t", [P, KC, S], F32, kind="ExternalOutput")

    with tile.TileContext(nc) as tc:
        with (
            tc.tile_pool(name="persist", bufs=1) as pp,
            tc.tile_pool(name="epool", bufs=6) as ep,
            tc.tile_pool(name="ctxp", bufs=3) as cp,
            tc.tile_pool(name="workp", bufs=2) as wp,
            tc.tile_pool(name="outp", bufs=3) as op_,
            tc.tile_pool(name="psA", bufs=4, space="PSUM") as psA,
            tc.tile_pool(name="psC", bufs=2, space="PSUM") as psC,
        ):
            # ---- loads ----
            x_sb = pp.tile([P, KC, S], F16)
            nc.sync.dma_start(out=x_sb, in_=xt[:, :, :])
            wq_sb = pp.tile([P, KC, DD], F16)
            nc.sync.dma_start(out=wq_sb, in_=wq[:, :, :])
            wk_sb = pp.tile([P, KC, DD], F16)
            nc.sync.dma_start(out=wk_sb, in_=wk[:, :, :])
            wv_sb = pp.tile([P, KC, DD], F16)
            nc.sync.dma_start(out=wv_sb, in_=wv[:, :, :])
            wo_sb = pp.tile([P, 4, D], F16)
            nc.sync.dma_start(out=wo_sb, in_=wo[:, :, :])
            bqk_sb = pp.tile([P, 6], F32)
            nc.sync.dma_start(out=bqk_sb, in_=bqk[:, :])
            bvr_sb = pp.tile([1, DD], F16)
            nc.sync.dma_start(out=bvr_sb, in_=bvr[:, :])
            msk_sb = pp.tile([P, 4, QT], F16)
            nc.sync.dma_start(out=msk_sb, in_=msk[:, :, :])

            ones_sb = pp.tile([1, P], F16)
            nc.vector.memset(ones_sb, 1.0)

            # V bias broadcast to all partitions: bvb[p, n] = bv[n]
            ps_bvb = psA.tile([P, QT], F32, tag="mm", name="ps_bvb")
            nc.tensor.matmul(
                ps_bvb[:, 0:DD], lhsT=ones_sb, rhs=bvr_sb, start=True, stop=True
            )
            bvb_sb = pp.tile([P, DD], F32)
            nc.vector.tensor_copy(bvb_sb, ps_bvb[:, 0:DD])

            # ---- Q^T and K^T projections: [384(3x128), 2048] fp16 ----
            qt_sb = pp.tile([P, MC, S], F16)
            kt_sb = pp.tile([P, MC, S], F16)
            for t, (w_sb, dst, boff) in enumerate(
                [(wq_sb, qt_sb, 0), (wk_sb, kt_sb, 3)]
            ):
                for m in range(MC):
                    for s in range(NQ):
                        ps = psA.tile([P, QT], F32, tag="mm", name=f"psp{t}_{m}_{s}")
                        for c in range(KC):
                            nc.tensor.matmul(
                                ps,
                                lhsT=w_sb[:, c, m * P : (m + 1) * P],
                                rhs=x_sb[:, c, s * QT : (s + 1) * QT],
                                start=(c == 0),
                                stop=(c == KC - 1),
                            )
                        nc.vector.tensor_scalar_add(
                            dst[:, m, s * QT : (s + 1) * QT],
                            ps,
                            bqk_sb[:, boff + m : boff + m + 1],
                        )

            # ---- V projection, seq-major with ones column per head ----
            # v_sb[:, i, h, :] = [V_d0:128 | ones | V_d128:192] for key-tile i
            v_sb = pp.tile([P, NK, 2, DH + 1], F16)
            for i in range(NK):
                ps = psA.tile([P, QT], F32, tag="mm", name=f"psv{i}")
                for c in range(KC):
                    nc.tensor.matmul(
                        ps[:, 0:DD],
                        lhsT=x_sb[:, c, i * P : (i + 1) * P],
                        rhs=wv_sb[:, c, :],
                        start=(c == 0),
                        stop=(c == KC - 1),
                    )
                for h in range(2):
                    nc.vector.tensor_add(
                        v_sb[:, i, h, 0:128],
                        ps[:, h * DH : h * DH + 128],
                        bvb_sb[:, h * DH : h * DH + 128],
                    )
                    nc.vector.tensor_add(
                        v_sb[:, i, h, 129 : DH + 1],
                        ps[:, h * DH + 128 : (h + 1) * DH],
                        bvb_sb[:, h * DH + 128 : (h + 1) * DH],
                    )
                    nc.vector.memset(v_sb[:, i, h, 128:129], 1.0)

            # ---- attention + output projection, per query tile ----
            for j in range(NQ):
                qs = slice(j * QT, (j + 1) * QT)
                nk_j = 4 * j + 4  # causal: key tiles 0 .. 4j+3 only
                cA = [
                    psC.tile([P, QT], F32, tag="cA", name=f"cA{h}_{j}")
                    for h in range(2)
                ]
                cB = [
                    psC.tile([65, QT], F32, tag="cB", name=f"cB{h}_{j}")
                    for h in range(2)
                ]
                for i in range(nk_j):
                    ks = slice(i * P, (i + 1) * P)
                    sps = [
                        psA.tile([P, QT], F32, tag="mm", name=f"sc{h}_{j}_{i}")
                        for h in range(2)
                    ]
                    # K=128 pieces, then the two K=64 pieces back-to-back
                    # (different PE row groups -> they overlap in the array)
                    for pi in range(2):
                        for h in range(2):
                            c, p0, pl = HEAD_PIECES[h][pi]
                            nc.tensor.matmul(
                                sps[h],
                                lhsT=kt_sb[p0 : p0 + pl, c, ks],
                                rhs=qt_sb[p0 : p0 + pl, c, qs],
                                start=(pi == 0),
                                stop=(pi == 1),
                            )
                    r = i - 4 * j
                    for h in range(2):
                        e = ep.tile([P, QT], F16, tag="e", name=f"e{h}_{j}_{i}")
                        nc.scalar.activation(e, sps[h], EXP, scale=SCALE)
                        if r >= 0:
                            nc.vector.tensor_mul(e, e, msk_sb[:, r, :])
                        nc.tensor.matmul(
                            cA[h],
                            lhsT=v_sb[:, i, h, 0:128],
                            rhs=e,
                            start=(i == 0),
                            stop=(i == nk_j - 1),
                        )
                        nc.tensor.matmul(
                            cB[h],
                            lhsT=v_sb[:, i, h, 128 : DH + 1],
                            rhs=e,
                            start=(i == 0),
                            stop=(i == nk_j - 1),
                        )
                # normalize: ctx / denom (denom = row 0 of cB)
                ctxA, ctxB = [], []
                for h in range(2):
                    rd = wp.tile([1, QT], F32, tag="rd", name=f"rd{h}_{j}")
                    nc.vector.reciprocal(rd, cB[h][0:1, :])
                    rdh = wp.tile([1, QT], F16, tag="rdh", name=f"rdh{h}_{j}")
                    nc.vector.tensor_copy(rdh, rd)
                    bps = psA.tile([P, QT], F32, tag="mm", name=f"bps{h}_{j}")
                    nc.tensor.matmul(bps, lhsT=ones_sb, rhs=rdh, start=True, stop=True)
                    bsb = wp.tile([P, QT], F32, tag="bsb", name=f"bsb{h}_{j}")
                    nc.vector.tensor_copy(bsb, bps)
                    cta = cp.tile([P, QT], F16, tag="ctA", name=f"ctA{h}_{j}")
                    nc.vector.tensor_mul(cta, cA[h], bsb)
                    ctb = cp.tile([65, QT], F16, tag="ctB", name=f"ctB{h}_{j}")
                    nc.vector.tensor_mul(ctb, cB[h][0:65, :], bsb[0:65, :])
                    ctxA.append(cta)
                    ctxB.append(ctb)
                # output projection for this query tile
                for m in range(KC):
                    ms = slice(m * P, (m + 1) * P)
                    po = psA.tile([P, QT], F32, tag="mm", name=f"po{m}_{j}")
                    nc.tensor.matmul(po, lhsT=wo_sb[:, 0, ms], rhs=ctxA[0], start=True, stop=False)
                    nc.tensor.matmul(po, lhsT=wo_sb[0:65, 1, ms], rhs=ctxB[0], start=False, stop=False)
                    nc.tensor.matmul(po, lhsT=wo_sb[:, 2, ms], rhs=ctxA[1], start=False, stop=False)
                    nc.tensor.matmul(po, lhsT=wo_sb[0:65, 3, ms], rhs=ctxB[1], start=False, stop=True)
                    osb = op_.tile([P, QT], F32, tag="osb", name=f"osb{m}_{j}")
                    nc.vector.tensor_copy(osb, po)
                    nc.sync.dma_start(out=out_t[:, m, qs], in_=osb)

    nc.compile()
    return nc


def _get_nc():
    if "nc" not in _CACHE:
        _CACHE["nc"] = _build_nc()
    return _CACHE["nc"]


def _masks():
    kk = np.arange(P)[:, None, None]
    r = np.arange(4)[None, :, None]
    qq = np.arange(QT)[None, None, :]
    return (qq >= kk + P * r).astype(np.float16)


def host_prep(x, Wq, bq, Wk, bk, Wv, bv, Wo):
    """Build the 8 per-core input maps (core c: batch c//2, head-pair c%2)."""
    f16 = np.float16
    x = np.asarray(x, dtype=np.float32)
    Wq, Wk, Wv, Wo = (np.asarray(a, dtype=np.float32) for a in (Wq, Wk, Wv, Wo))
    bq, bk, bv = (np.asarray(a, dtype=np.float32) for a in (bq, bk, bv))
    masks = _masks()
    xt16 = {}
    for b in range(4):
        xt16[b] = np.ascontiguousarray(
            x[b].T.reshape(KC, P, S).transpose(1, 0, 2)
        ).astype(f16)
    in_maps = []
    for c in range(8):
        b, hp = divmod(c, 2)
        cs = slice(hp * DD, (hp + 1) * DD)
        wq16 = np.ascontiguousarray(
            Wq[:, cs].reshape(KC, P, DD).transpose(1, 0, 2)
        ).astype(f16)
        wk16 = np.ascontiguousarray(
            Wk[:, cs].reshape(KC, P, DD).transpose(1, 0, 2)
        ).astype(f16)
        wv16 = np.ascontiguousarray(
            Wv[:, cs].reshape(KC, P, DD).transpose(1, 0, 2)
        ).astype(f16)
        wo_s = Wo[cs, :]
        woc = np.zeros((P, 4, D), np.float32)
        woc[:, 0, :] = wo_s[0:128]
        woc[1:65, 1, :] = wo_s[128:192]
        woc[:, 2, :] = wo_s[192:320]
        woc[1:65, 3, :] = wo_s[320:384]
        bqk_c = np.concatenate(
            [bq[cs].reshape(MC, P).T, bk[cs].reshape(MC, P).T], axis=1
        ).astype(np.float32)
        in_maps.append(
            {
                "xt": xt16[b],
                "wq": wq16,
                "wk": wk16,
                "wv": wv16,
                "wo": woc.astype(f16),
                "bqk": np.ascontiguousarray(bqk_c),
                "bvr": np.ascontiguousarray(bv[cs].reshape(1, DD)).astype(f16),
                "msk": masks,
            }
        )
    return in_maps


def combine(per_core_out, bo):
    """Sum the per-batch core pairs and undo the transposed layout."""
    bo = np.asarray(bo, dtype=np.float32)
    out = np.empty((4, S, D), np.float32)
    for b in range(4):
        pt = per_core_out[2 * b] + per_core_out[2 * b + 1]  # [P, KC, S]
        out[b] = pt.transpose(1, 0, 2).reshape(D, S).T + bo
    return out


def run(inp, trace=False):
    from concourse.bass_utils import run_bass_kernel_spmd

    nc = _get_nc()
    in_maps = host_prep(
        inp["inputs"], inp["Wq"], inp["bq"], inp["Wk"], inp["bk"],
        inp["Wv"], inp["bv"], inp["Wo"],
    )
    kw = {}
    if trace:
        kw = dict(trace=True, trace_cores=list(range(8)))
    res = run_bass_kernel_spmd(nc, in_maps, core_ids=list(range(8)), **kw)
    out = combine([r["out_t"] for r in res.results], inp["bo"])
    return out, res


def kernel(inputs, Wq, bq, Wk, bk, Wv, bv, Wo, bo):
    out, _ = run(
        {"inputs": inputs, "Wq": Wq, "bq": bq, "Wk": Wk, "bk": bk,
         "Wv": Wv, "bv": bv, "Wo": Wo, "bo": bo}
    )
    return out
